# revision 6
# baseline (speedup 1.0000x reference)
"""GCNConv Trainium2 kernel: sigmoid(segment_sum(edge_val * (X@W)[edge_col], edge_row) + bias).

Uses the reassociation A@(XW) = (A@X)W:
  - Shard destination rows across 8 cores (12500 rows each); edges partitioned
    by dest row (edge_row is sorted). One NEFF runs SPMD on all 8 cores;
    per-core behavior differs only through input data.
  - Dest rows processed in SUPERWINDOWS of 256 rows (49 per core). Gathers are
    batched per (group of 4 superwindows, node-quadrant): 52 dma_gather calls
    per core (vs one per window-cell) to amortize the ~1us SWDGE fixed cost.
  - X is cast to fp16 on host: 256B gather descriptors, 1-cycle/row PE
    matmuls, half the SBUF footprint. Precision: fp16 RN err ~1e-4 rel.
  - Per 128-edge slot, build a val-scaled one-hot over the 256 superwindow
    dests with ONE DVE tensor_scalar (is_equal + mult, fp16 out, 4x_2p mode):
    OH[p,r] = (iota256[r] == key[p]) * val[p], then scatter-add via PE matmul
    lhsT=gathered-X-rows, rhs=OH accumulating the TRANSPOSED superwindow
    aggregate [feat, 256 dests] in PSUM.
  - Superwindow epilogue: copy PSUM->SBUF fp16, z = matmul(lhsT=W_fp16, rhs),
    then one ACT op computes sigmoid(z + bias) with bias on the partition
    axis, DMA out to a transposed [128, 12544] output; host transposes once.
"""
import sys

sys.path.insert(0, "/opt/trn_rl_repo")

import numpy as np

import concourse.mybir as mybir
import concourse.tile as tile
from concourse import bacc
import concourse.bass_utils as bass_utils

# Problem constants (contest contract)
N_NODES = 100000
F = 128
NCORES = 8
ROWS_PER_CORE = N_NODES // NCORES          # 12500
P = 128
SW = 256                                   # dest rows per superwindow
N_SW = -(-ROWS_PER_CORE // SW)             # 49 superwindows/core (last = 212)
GW = 4                                     # superwindows per gather group
N_GRP = -(-N_SW // GW)                     # 13 groups (last has 1 SW)
N_QUAD = 4
QUAD = N_NODES // N_QUAD                   # 25000 (< int16 max)
GBUFS = 8                                  # gathered tiles in flight (2 groups)
NQUEUES = 4
ACT_FRAC = 0.45                            # fraction of slots val-scaled on ACT

_cache = {}


def _build_program(C2):
    """Build + compile the SPMD program. C2: [N_SW][N_QUAD] slots per cell
    (identical across cores). Returns compiled Bacc."""
    dt = mybir.dt
    C2 = [list(map(int, row)) for row in C2]
    S2 = sum(sum(row) for row in C2)           # total 128-edge slots per core
    # group capacities: slots per (group, quadrant) gather call
    grp_sws = [list(range(g * GW, min((g + 1) * GW, N_SW))) for g in range(N_GRP)]
    cap_gq = [[sum(C2[sw][q] for sw in sws) for q in range(N_QUAD)]
              for g, sws in enumerate(grp_sws)]
    CMAX = max(max(row) for row in cap_gq)
    # slot base of cell (sw, q) in the flat (g, q, sw)-ordered slot space
    slot_base = {}
    s = 0
    for g, sws in enumerate(grp_sws):
        for q in range(N_QUAD):
            for sw in sws:
                slot_base[(sw, q)] = s
                s += C2[sw][q]
    assert s == S2

    nc = bacc.Bacc("TRN2", target_bir_lowering=False, debug=False,
                   enable_asserts=False, num_devices=NCORES,
                   num_swdge_queues=NQUEUES)

    x_d = nc.dram_tensor("x", [N_NODES, F], dt.float16, kind="ExternalInput")
    w_d = nc.dram_tensor("w", [F, F], dt.float16, kind="ExternalInput")
    bias_d = nc.dram_tensor("bias", [F, 1], dt.float32, kind="ExternalInput")
    iota_d = nc.dram_tensor("iota", [P, SW], dt.float16, kind="ExternalInput")
    gidx_d = nc.dram_tensor("gidx", [P, S2 * 8], dt.int16, kind="ExternalInput")
    key_d = nc.dram_tensor("key", [P, S2], dt.float32, kind="ExternalInput")
    val_d = nc.dram_tensor("val", [P, S2], dt.float32, kind="ExternalInput")
    yt_d = nc.dram_tensor("yt", [F, N_SW * SW], dt.float32, kind="ExternalOutput")

    with tile.TileContext(nc) as tc:
        with (
            tc.tile_pool(name="cst", bufs=1) as cst,
            tc.tile_pool(name="sbg", bufs=GBUFS) as sbg,
            tc.tile_pool(name="sbo", bufs=8) as sbo,
            tc.tile_pool(name="sbs", bufs=8) as sbs,
            tc.tile_pool(name="sby", bufs=4) as sby,
            tc.tile_pool(name="psw", bufs=4, space="PSUM") as psw,
            tc.tile_pool(name="psz", bufs=2, space="PSUM") as psz,
        ):
            iota_t = cst.tile([P, SW], dt.float16)
            nc.sync.dma_start(iota_t[:], iota_d[:])
            w_t = cst.tile([F, F], dt.float16)
            nc.sync.dma_start(w_t[:], w_d[:])
            bias_t = cst.tile([F, 1], dt.float32)
            nc.sync.dma_start(bias_t[:], bias_d[:])
            gidx_t = cst.tile([P, S2 * 8], dt.int16)
            nc.sync.dma_start(gidx_t[:], gidx_d[:])
            key_t = cst.tile([P, S2], dt.float32)
            nc.sync.dma_start(key_t[:], key_d[:])
            val_t = cst.tile([P, S2], dt.float32)
            nc.sync.dma_start(val_t[:], val_d[:])

            qn = 0
            for g, sws in enumerate(grp_sws):
                gt = []
                for q in range(N_QUAD):
                    cap = cap_gq[g][q]
                    s0 = slot_base[(sws[0], q)]
                    gq = sbg.tile([P, CMAX, F], dt.float16, tag="g")
                    nc.gpsimd.dma_gather(
                        out_ap=gq[:, :cap, :],
                        in_ap=x_d[q * QUAD : (q + 1) * QUAD, :],
                        idxs_ap=gidx_t[:, s0 * 8 : (s0 + cap) * 8],
                        num_idxs=cap * P,
                        num_idxs_reg=cap * P,
                        elem_size=F,
                        single_packet=False,
                        queue_num=qn % NQUEUES,
                    )
                    qn += 1
                    gt.append(gq)
                for sw in sws:
                    nslots = sum(C2[sw][q] for q in range(N_QUAD))
                    pw = psw.tile([F, SW], dt.float32, tag="pw")
                    j = 0
                    for q in range(N_QUAD):
                        s0 = slot_base[(sw, q)]
                        k0 = s0 - slot_base[(sws[0], q)]
                        for k in range(C2[sw][q]):
                            oh = sbo.tile([P, SW], dt.float16, tag="oh")
                            # route a fraction of slots through ACT: val-scale
                            # the gathered rows there, DVE builds binary 1-hot
                            if (j * 100) % nslots < int(ACT_FRAC * 100):
                                gs = sbs.tile([P, F], dt.float16, tag="gs")
                                nc.scalar.activation(
                                    gs[:], gt[q][:, k0 + k, :],
                                    mybir.ActivationFunctionType.Copy,
                                    scale=val_t[:, s0 + k : s0 + k + 1],
                                )
                                nc.vector.tensor_scalar(
                                    out=oh[:],
                                    in0=iota_t[:],
                                    scalar1=key_t[:, s0 + k : s0 + k + 1],
                                    scalar2=None,
                                    op0=mybir.AluOpType.is_equal,
                                )
                                lhs = gs[:]
                            else:
                                nc.vector.tensor_scalar(
                                    out=oh[:],
                                    in0=iota_t[:],
                                    scalar1=key_t[:, s0 + k : s0 + k + 1],
                                    scalar2=val_t[:, s0 + k : s0 + k + 1],
                                    op0=mybir.AluOpType.is_equal,
                                    op1=mybir.AluOpType.mult,
                                )
                                lhs = gt[q][:, k0 + k, :]
                            nc.tensor.matmul(
                                pw[:], lhsT=lhs, rhs=oh[:],
                                start=(j == 0), stop=(j == nslots - 1),
                            )
                            j += 1
                    at = sby.tile([F, SW], dt.float16, tag="at")
                    nc.scalar.activation(at[:], pw[:],
                                         mybir.ActivationFunctionType.Copy)
                    z = psz.tile([F, SW], dt.float32, tag="z")
                    nc.tensor.matmul(z[:], lhsT=w_t[:], rhs=at[:],
                                     start=True, stop=True)
                    ys = sby.tile([F, SW], dt.float32, tag="ys")
                    nc.scalar.activation(ys[:], z[:],
                                         mybir.ActivationFunctionType.Sigmoid,
                                         bias=bias_t[:, 0:1])
                    nc.sync.dma_start(yt_d[:, sw * SW : (sw + 1) * SW], ys[:])

    nc.compile()
    return nc


def _preprocess(X, edge_row, edge_col, edge_val, weight, bias):
    edge_row = np.asarray(edge_row, dtype=np.int64)
    edge_col = np.asarray(edge_col, dtype=np.int64)
    edge_val = np.asarray(edge_val, dtype=np.float32)

    if not np.all(edge_row[:-1] <= edge_row[1:]):
        o = np.argsort(edge_row, kind="stable")
        edge_row, edge_col, edge_val = edge_row[o], edge_col[o], edge_val[o]

    core = edge_row // ROWS_PER_CORE
    rl = edge_row % ROWS_PER_CORE
    sw = rl // SW
    dkey = (rl - sw * SW).astype(np.float32)
    quad = edge_col // QUAD
    cidx = (edge_col % QUAD).astype(np.int16)
    grp = sw // GW

    # cell = (core, sw, q); order cells (g, q, sw) within each core so each
    # (group, quadrant) gather call covers a contiguous slot range
    ncells = NCORES * N_SW * N_QUAD
    cell_id = (core * N_SW + sw) * N_QUAD + quad
    counts = np.bincount(cell_id, minlength=ncells).reshape(NCORES, N_SW, N_QUAD)

    # slots per (sw, q): max over cores so the program is SPMD-safe
    C2 = -(-counts.max(axis=0) // P)             # [N_SW, N_QUAD]
    C2 = np.maximum(C2, 1)

    # flat slot layout in (g, q, sw) order
    order_cells = []
    for g in range(N_GRP):
        sws = range(g * GW, min((g + 1) * GW, N_SW))
        for q in range(N_QUAD):
            for w in sws:
                order_cells.append((w, q))
    slot_base = np.zeros((N_SW, N_QUAD), dtype=np.int64)
    s = 0
    for (w, q) in order_cells:
        slot_base[w, q] = s
        s += C2[w, q]
    S2 = int(s)

    # sort edges by (core, cell, col) -- col-sort inside cells for DRAM
    # locality of the gather descriptors
    skey = (cell_id * QUAD + (edge_col % QUAD)) + 0
    order = np.argsort(core * (N_SW * N_QUAD * QUAD) + skey, kind="stable")
    core_s = core[order]
    cell_s = cell_id[order] % (N_SW * N_QUAD)
    w_s = cell_s // N_QUAD
    q_s = cell_s % N_QUAD

    cstarts = np.zeros(ncells, dtype=np.int64)
    cstarts[1:] = np.cumsum(counts.reshape(-1))[:-1]
    rank = np.arange(len(order), dtype=np.int64) - cstarts[cell_id[order]]
    pos = slot_base[w_s, q_s] * P + rank         # flat idx position within core

    key_flat = np.zeros((NCORES, S2 * P), dtype=np.float32)
    val_flat = np.zeros((NCORES, S2 * P), dtype=np.float32)
    gidx_flat = np.zeros((NCORES, S2 * P), dtype=np.int16)
    key_flat[core_s, pos] = dkey[order]
    val_flat[core_s, pos] = edge_val[order]
    gidx_flat[core_s, pos] = cidx[order]

    key_m = np.ascontiguousarray(key_flat.reshape(NCORES, S2, P).transpose(0, 2, 1))
    val_m = np.ascontiguousarray(val_flat.reshape(NCORES, S2, P).transpose(0, 2, 1))
    # wrapped gather-index layout: within each slot-region, idx j -> partition
    # j%16 (replicated across the 8 16-partition groups), free column j//16.
    gw_ = gidx_flat.reshape(NCORES, S2, 8, 16).transpose(0, 1, 3, 2)  # [NC,S2,16,8]
    gw_ = np.tile(gw_, (1, 1, 8, 1))                                  # [NC,S2,128,8]
    gidx_m = np.ascontiguousarray(gw_.transpose(0, 2, 1, 3).reshape(NCORES, P, S2 * 8))

    iota = np.tile(np.arange(SW, dtype=np.float16)[None, :], (P, 1))
    shared = {
        "x": np.ascontiguousarray(np.asarray(X, dtype=np.float16)),
        "w": np.ascontiguousarray(np.asarray(weight, dtype=np.float16)),
        "bias": np.ascontiguousarray(np.asarray(bias, dtype=np.float32)[:, None]),
        "iota": iota,
    }
    per_core = [
        {"gidx": np.ascontiguousarray(gidx_m[c]),
         "key": np.ascontiguousarray(key_m[c]),
         "val": np.ascontiguousarray(val_m[c])}
        for c in range(NCORES)
    ]
    return C2, shared, per_core


def kernel(X, edge_row, edge_col, edge_val, weight, bias):
    C2, shared, per_core = _preprocess(X, edge_row, edge_col, edge_val,
                                       weight, bias)
    ckey = C2.tobytes()
    if ckey not in _cache:
        _cache[ckey] = _build_program(C2)
    nc = _cache[ckey]

    in_maps = [dict(shared, **per_core[c]) for c in range(NCORES)]
    res = bass_utils.run_bass_kernel_spmd(nc, in_maps, core_ids=list(range(NCORES)))

    out = np.empty((N_NODES, F), dtype=np.float32)
    for c in range(NCORES):
        out[c * ROWS_PER_CORE : (c + 1) * ROWS_PER_CORE] = \
            res.results[c]["yt"].T[:ROWS_PER_CORE]
    return out


# revision 7
# speedup vs baseline: 2.2121x; 2.2121x over previous
"""GCNConv Trainium2 kernel: sigmoid(segment_sum(edge_val * (X@W)[edge_col], edge_row) + bias).

Uses the reassociation A@(XW) = (A@X)W:
  - Shard destination rows across 8 cores (12500 rows each); edges partitioned
    by dest row (edge_row is sorted). One NEFF runs SPMD on all 8 cores.
  - Dest rows in 128-row windows (98/core). Gathers batched per (group of 6
    windows, node-quadrant): 68 dma_gather calls/core amortize the ~1us SWDGE
    fixed cost. X cast to fp16 on host: 256B descriptors, 1-cycle/row PE.
  - One-hot build is BATCHED on the DVE: per (group, quadrant) region, ONE
    tensor_tensor is_equal + ONE mult build all slots' val-scaled one-hots.
    Per-slot keys/vals enter via a replicated-x4 fp16 layout [P, S2, 4] read
    through a [cap][32 x stride-0][4 x stride-1] access pattern - last dim
    packed so the DVE 2x_1p fp16 mode can engage (vs 1x for broadcasts).
  - Per 128-edge slot, PE matmul lhsT=gathered-rows, rhs=one-hot accumulates
    the TRANSPOSED window aggregate [feat, 128 dests] in PSUM.
  - Window epilogue: copy PSUM->SBUF fp16, z = matmul(lhsT=W_fp16, rhs), one
    ACT computes sigmoid(z + bias), DMA to transposed [128, 12544] output;
    host transposes once.
"""
import sys

sys.path.insert(0, "/opt/trn_rl_repo")

import numpy as np

import concourse.mybir as mybir
import concourse.tile as tile
from concourse import bacc
import concourse.bass_utils as bass_utils

# Problem constants (contest contract)
N_NODES = 100000
F = 128
NCORES = 8
ROWS_PER_CORE = N_NODES // NCORES          # 12500
P = 128
N_WIN = -(-ROWS_PER_CORE // P)             # 98 windows/core (last covers 84)
GW = 6                                     # windows per gather group
N_GRP = -(-N_WIN // GW)                    # 17 groups (last has 2 windows)
N_QUAD = 4
QUAD = N_NODES // N_QUAD                   # 25000 (< int16 max)
GBUFS = 8                                  # gathered tiles in flight (2 groups)
OHBUFS = 4
NQUEUES = 4

_cache = {}


def _build_program(C):
    """Build + compile the SPMD program. C: [N_WIN][N_QUAD] slots per cell
    (identical across cores)."""
    dt = mybir.dt
    C = [list(map(int, row)) for row in C]
    S2 = sum(sum(row) for row in C)
    grp_ws = [list(range(g * GW, min((g + 1) * GW, N_WIN))) for g in range(N_GRP)]
    cap_gq = [[sum(C[w][q] for w in ws) for q in range(N_QUAD)]
              for g, ws in enumerate(grp_ws)]
    CMAX = max(max(row) for row in cap_gq)
    slot_base = {}
    s = 0
    for g, ws in enumerate(grp_ws):
        for q in range(N_QUAD):
            for w in ws:
                slot_base[(w, q)] = s
                s += C[w][q]
    assert s == S2

    nc = bacc.Bacc("TRN2", target_bir_lowering=False, debug=False,
                   enable_asserts=False, num_devices=NCORES,
                   num_swdge_queues=NQUEUES)

    x_d = nc.dram_tensor("x", [N_NODES, F], dt.float16, kind="ExternalInput")
    w_d = nc.dram_tensor("w", [F, F], dt.float16, kind="ExternalInput")
    bias_d = nc.dram_tensor("bias", [F, 1], dt.float32, kind="ExternalInput")
    iota_d = nc.dram_tensor("iota", [P, 32, 4], dt.float16, kind="ExternalInput")
    gidx_d = nc.dram_tensor("gidx", [P, S2 * 8], dt.int16, kind="ExternalInput")
    krep_d = nc.dram_tensor("krep", [P, S2, 4], dt.float16, kind="ExternalInput")
    vrep_d = nc.dram_tensor("vrep", [P, S2, 4], dt.float16, kind="ExternalInput")
    yt_d = nc.dram_tensor("yt", [F, N_WIN * P], dt.float32, kind="ExternalOutput")

    with tile.TileContext(nc) as tc:
        with (
            tc.tile_pool(name="cst", bufs=1) as cst,
            tc.tile_pool(name="sbg", bufs=GBUFS) as sbg,
            tc.tile_pool(name="sbo", bufs=OHBUFS) as sbo,
            tc.tile_pool(name="sby", bufs=4) as sby,
            tc.tile_pool(name="psw", bufs=4, space="PSUM") as psw,
            tc.tile_pool(name="psz", bufs=2, space="PSUM") as psz,
        ):
            iota_t = cst.tile([P, 32, 4], dt.float16)
            nc.sync.dma_start(iota_t[:], iota_d[:])
            w_t = cst.tile([F, F], dt.float16)
            nc.sync.dma_start(w_t[:], w_d[:])
            bias_t = cst.tile([F, 1], dt.float32)
            nc.sync.dma_start(bias_t[:], bias_d[:])
            gidx_t = cst.tile([P, S2 * 8], dt.int16)
            nc.sync.dma_start(gidx_t[:], gidx_d[:])
            krep_t = cst.tile([P, S2, 4], dt.float16)
            nc.sync.dma_start(krep_t[:], krep_d[:])
            vrep_t = cst.tile([P, S2, 4], dt.float16)
            nc.sync.dma_start(vrep_t[:], vrep_d[:])

            qn = 0
            for g, ws in enumerate(grp_ws):
                gt = []
                oht = []
                for q in range(N_QUAD):
                    cap = cap_gq[g][q]
                    s0 = slot_base[(ws[0], q)]
                    gq = sbg.tile([P, CMAX, F], dt.float16, tag="g")
                    nc.gpsimd.dma_gather(
                        out_ap=gq[:, :cap, :],
                        in_ap=x_d[q * QUAD : (q + 1) * QUAD, :],
                        idxs_ap=gidx_t[:, s0 * 8 : (s0 + cap) * 8],
                        num_idxs=cap * P,
                        num_idxs_reg=cap * P,
                        elem_size=F,
                        single_packet=False,
                        queue_num=qn % NQUEUES,
                    )
                    qn += 1
                    gt.append(gq)
                    # batched one-hot build for the whole (group, quadrant)
                    # region: oh[p, k, d] = (iota[d] == key[p, s0+k]) * val
                    oh = sbo.tile([P, CMAX, 32, 4], dt.float16, tag="oh")
                    iota_b = iota_t[:, None, :, :].to_broadcast([P, cap, 32, 4])
                    key_b = krep_t[:, s0 : s0 + cap, None, :] \
                        .to_broadcast([P, cap, 32, 4])
                    val_b = vrep_t[:, s0 : s0 + cap, None, :] \
                        .to_broadcast([P, cap, 32, 4])
                    nc.vector.tensor_tensor(out=oh[:, :cap, :, :], in0=iota_b,
                                            in1=key_b,
                                            op=mybir.AluOpType.is_equal)
                    nc.vector.tensor_tensor(out=oh[:, :cap, :, :],
                                            in0=oh[:, :cap, :, :], in1=val_b,
                                            op=mybir.AluOpType.mult)
                    oht.append(oh)
                for w in ws:
                    nslots = sum(C[w][q] for q in range(N_QUAD))
                    pw = psw.tile([F, P], dt.float32, tag="pw")
                    j = 0
                    for q in range(N_QUAD):
                        s0 = slot_base[(w, q)]
                        k0 = s0 - slot_base[(ws[0], q)]
                        for k in range(C[w][q]):
                            nc.tensor.matmul(
                                pw[:], lhsT=gt[q][:, k0 + k, :],
                                rhs=oht[q][:, k0 + k, :, :],
                                start=(j == 0), stop=(j == nslots - 1),
                            )
                            j += 1
                    at = sby.tile([F, P], dt.float16, tag="at")
                    nc.scalar.activation(at[:], pw[:],
                                         mybir.ActivationFunctionType.Copy)
                    z = psz.tile([F, P], dt.float32, tag="z")
                    nc.tensor.matmul(z[:], lhsT=w_t[:], rhs=at[:],
                                     start=True, stop=True)
                    ys = sby.tile([F, P], dt.float32, tag="ys")
                    nc.scalar.activation(ys[:], z[:],
                                         mybir.ActivationFunctionType.Sigmoid,
                                         bias=bias_t[:, 0:1])
                    nc.sync.dma_start(yt_d[:, w * P : (w + 1) * P], ys[:])

    nc.compile()
    return nc


def _preprocess(X, edge_row, edge_col, edge_val, weight, bias):
    edge_row = np.asarray(edge_row, dtype=np.int64)
    edge_col = np.asarray(edge_col, dtype=np.int64)
    edge_val = np.asarray(edge_val, dtype=np.float32)

    if not np.all(edge_row[:-1] <= edge_row[1:]):
        o = np.argsort(edge_row, kind="stable")
        edge_row, edge_col, edge_val = edge_row[o], edge_col[o], edge_val[o]

    core = edge_row // ROWS_PER_CORE
    rl = edge_row % ROWS_PER_CORE
    win = rl // P
    dkey = (rl - win * P).astype(np.float16)
    quad = edge_col // QUAD
    cidx = (edge_col % QUAD).astype(np.int16)

    ncells = NCORES * N_WIN * N_QUAD
    cell_id = (core * N_WIN + win) * N_QUAD + quad
    counts = np.bincount(cell_id, minlength=ncells).reshape(NCORES, N_WIN, N_QUAD)

    C = -(-counts.max(axis=0) // P)              # [N_WIN, N_QUAD]
    C = np.maximum(C, 1)

    order_cells = []
    for g in range(N_GRP):
        ws = range(g * GW, min((g + 1) * GW, N_WIN))
        for q in range(N_QUAD):
            for w in ws:
                order_cells.append((w, q))
    slot_base = np.zeros((N_WIN, N_QUAD), dtype=np.int64)
    s = 0
    for (w, q) in order_cells:
        slot_base[w, q] = s
        s += C[w, q]
    S2 = int(s)

    # sort edges by (core, cell, col); col-sort gives the gather descriptors
    # ascending DRAM addresses within each cell
    skey = cell_id * QUAD + (edge_col % QUAD)
    order = np.argsort(skey, kind="stable")
    cell_s = cell_id[order] % (N_WIN * N_QUAD)
    core_s = core[order]
    w_s = cell_s // N_QUAD
    q_s = cell_s % N_QUAD

    cstarts = np.zeros(ncells, dtype=np.int64)
    cstarts[1:] = np.cumsum(counts.reshape(-1))[:-1]
    rank = np.arange(len(order), dtype=np.int64) - cstarts[cell_id[order]]
    pos = slot_base[w_s, q_s] * P + rank         # flat idx position within core

    key_flat = np.zeros((NCORES, S2 * P), dtype=np.float16)
    val_flat = np.zeros((NCORES, S2 * P), dtype=np.float16)
    gidx_flat = np.zeros((NCORES, S2 * P), dtype=np.int16)
    key_flat[core_s, pos] = dkey[order]
    val_flat[core_s, pos] = edge_val[order].astype(np.float16)
    gidx_flat[core_s, pos] = cidx[order]

    # [NC, S2*P] -> [NC, P, S2] -> replicate x4 -> [NC, P, S2, 4]
    key_m = key_flat.reshape(NCORES, S2, P).transpose(0, 2, 1)
    val_m = val_flat.reshape(NCORES, S2, P).transpose(0, 2, 1)
    krep = np.ascontiguousarray(np.repeat(key_m[:, :, :, None], 4, axis=3))
    vrep = np.ascontiguousarray(np.repeat(val_m[:, :, :, None], 4, axis=3))

    gw_ = gidx_flat.reshape(NCORES, S2, 8, 16).transpose(0, 1, 3, 2)
    gw_ = np.tile(gw_, (1, 1, 8, 1))
    gidx_m = np.ascontiguousarray(gw_.transpose(0, 2, 1, 3).reshape(NCORES, P, S2 * 8))

    iota = np.tile(np.arange(P, dtype=np.float16).reshape(1, 32, 4), (P, 1, 1))
    shared = {
        "x": np.ascontiguousarray(np.asarray(X, dtype=np.float16)),
        "w": np.ascontiguousarray(np.asarray(weight, dtype=np.float16)),
        "bias": np.ascontiguousarray(np.asarray(bias, dtype=np.float32)[:, None]),
        "iota": np.ascontiguousarray(iota),
    }
    per_core = [
        {"gidx": gidx_m[c], "krep": krep[c], "vrep": vrep[c]}
        for c in range(NCORES)
    ]
    return C, shared, per_core


def kernel(X, edge_row, edge_col, edge_val, weight, bias):
    C, shared, per_core = _preprocess(X, edge_row, edge_col, edge_val,
                                      weight, bias)
    ckey = C.tobytes()
    if ckey not in _cache:
        _cache[ckey] = _build_program(C)
    nc = _cache[ckey]

    in_maps = [dict(shared, **per_core[c]) for c in range(NCORES)]
    res = bass_utils.run_bass_kernel_spmd(nc, in_maps, core_ids=list(range(NCORES)))

    out = np.empty((N_NODES, F), dtype=np.float32)
    for c in range(NCORES):
        out[c * ROWS_PER_CORE : (c + 1) * ROWS_PER_CORE] = \
            res.results[c]["yt"].T[:ROWS_PER_CORE]
    return out


# revision 12
# speedup vs baseline: 2.6981x; 1.2197x over previous
"""GCNConv Trainium2 kernel: sigmoid(segment_sum(edge_val * (X@W)[edge_col], edge_row) + bias).

Uses the reassociation A@(XW) = (A@X)W:
  - Shard destination rows across 8 cores (12500 rows each); edges partitioned
    by dest row (edge_row is sorted). One NEFF runs SPMD on all 8 cores.
  - Dest rows in 128-row windows (98/core). Gathers batched per (group of 6
    windows, node-quadrant): 68 dma_gather calls/core amortize the ~1us SWDGE
    fixed cost. X cast to fp16 on host: 256B descriptors, 1-cycle/row PE.
  - One-hot build is BATCHED on the DVE: per (group, quadrant) region, ONE
    tensor_tensor is_equal + ONE mult build all slots' val-scaled one-hots.
    Per-slot keys/vals enter via a replicated-x4 fp16 layout [P, S2, 4] read
    through a [cap][32 x stride-0][4 x stride-1] access pattern - last dim
    packed so the DVE 2x_1p fp16 mode can engage (vs 1x for broadcasts).
  - Per 128-edge slot, PE matmul lhsT=gathered-rows, rhs=one-hot accumulates
    the TRANSPOSED window aggregate [feat, 128 dests] in PSUM.
  - Window epilogue: copy PSUM->SBUF fp16, z = matmul(lhsT=W_fp16, rhs), one
    ACT computes sigmoid(z + bias), DMA to transposed [128, 12544] output;
    host transposes once.
"""
import sys

sys.path.insert(0, "/opt/trn_rl_repo")

import numpy as np

import concourse.mybir as mybir
import concourse.tile as tile
from concourse import bacc
import concourse.bass_utils as bass_utils

# Problem constants (contest contract)
N_NODES = 100000
F = 128
NCORES = 8
ROWS_PER_CORE = N_NODES // NCORES          # 12500
P = 128
N_WIN = -(-ROWS_PER_CORE // P)             # 98 windows/core (last covers 84)
GW = 3                                     # windows per gather group
N_GRP = -(-N_WIN // GW)                    # 33 groups
N_QUAD = 4
QUAD = N_NODES // N_QUAD                   # 25000 (< int16 max)
GBUFS = 8                                  # gathered tiles in flight (2 groups)
OHBUFS = 4
NQUEUES = 4

_cache = {}


def _build_program(C):
    """Build + compile the SPMD program. C: [N_WIN][N_QUAD] slots per cell
    (identical across cores)."""
    dt = mybir.dt
    C = [list(map(int, row)) for row in C]
    S2 = sum(sum(row) for row in C)
    grp_ws = [list(range(g * GW, min((g + 1) * GW, N_WIN))) for g in range(N_GRP)]
    cap_gq = [[sum(C[w][q] for w in ws) for q in range(N_QUAD)]
              for g, ws in enumerate(grp_ws)]
    CMAX = max(max(row) for row in cap_gq)
    slot_base = {}
    s = 0
    for g, ws in enumerate(grp_ws):
        for q in range(N_QUAD):
            for w in ws:
                slot_base[(w, q)] = s
                s += C[w][q]
    assert s == S2

    nc = bacc.Bacc("TRN2", target_bir_lowering=False, debug=False,
                   enable_asserts=False, num_devices=NCORES,
                   num_swdge_queues=NQUEUES)

    x_d = nc.dram_tensor("x", [N_NODES, F], dt.float16, kind="ExternalInput")
    w_d = nc.dram_tensor("w", [F, F], dt.float16, kind="ExternalInput")
    bias_d = nc.dram_tensor("bias", [F, 1], dt.float32, kind="ExternalInput")
    iota_d = nc.dram_tensor("iota", [P, 32, 4], dt.float16, kind="ExternalInput")
    gidx_d = nc.dram_tensor("gidx", [P, S2 * 8], dt.int16, kind="ExternalInput")
    krep_d = nc.dram_tensor("krep", [P, S2, 4], dt.float16, kind="ExternalInput")
    vrep_d = nc.dram_tensor("vrep", [P, S2, 4], dt.float16, kind="ExternalInput")
    yt_d = nc.dram_tensor("yt", [F, N_WIN * P], dt.float32, kind="ExternalOutput")

    with tile.TileContext(nc) as tc:
        with (
            tc.tile_pool(name="cst", bufs=1) as cst,
            tc.tile_pool(name="sbg", bufs=GBUFS) as sbg,
            tc.tile_pool(name="sbo", bufs=OHBUFS) as sbo,
            tc.tile_pool(name="sby", bufs=4) as sby,
            tc.tile_pool(name="psw", bufs=4, space="PSUM") as psw,
            tc.tile_pool(name="psz", bufs=2, space="PSUM") as psz,
        ):
            iota_t = cst.tile([P, 32, 4], dt.float16)
            nc.sync.dma_start(iota_t[:], iota_d[:])
            w_t = cst.tile([F, F], dt.float16)
            nc.sync.dma_start(w_t[:], w_d[:])
            bias_t = cst.tile([F, 1], dt.float32)
            nc.sync.dma_start(bias_t[:], bias_d[:])
            gidx_t = cst.tile([P, S2 * 8], dt.int16)
            nc.sync.dma_start(gidx_t[:], gidx_d[:])
            krep_t = cst.tile([P, S2, 4], dt.float16)
            nc.sync.dma_start(krep_t[:], krep_d[:])
            vrep_t = cst.tile([P, S2, 4], dt.float16)
            nc.sync.dma_start(vrep_t[:], vrep_d[:])

            qn = 0
            for g, ws in enumerate(grp_ws):
                gt = []
                oht = []
                for q in range(N_QUAD):
                    cap = cap_gq[g][q]
                    s0 = slot_base[(ws[0], q)]
                    gq = sbg.tile([P, CMAX, F], dt.float16, tag="g")
                    nc.gpsimd.dma_gather(
                        out_ap=gq[:, :cap, :],
                        in_ap=x_d[q * QUAD : (q + 1) * QUAD, :],
                        idxs_ap=gidx_t[:, s0 * 8 : (s0 + cap) * 8],
                        num_idxs=cap * P,
                        num_idxs_reg=cap * P,
                        elem_size=F,
                        single_packet=False,
                        queue_num=qn % NQUEUES,
                    )
                    qn += 1
                    gt.append(gq)
                    # batched one-hot build for the whole (group, quadrant)
                    # region: oh[p, k, d] = (iota[d] == key[p, s0+k]) * val
                    oh = sbo.tile([P, CMAX, 32, 4], dt.float16, tag="oh")
                    iota_b = iota_t[:, None, :, :].to_broadcast([P, cap, 32, 4])
                    key_b = krep_t[:, s0 : s0 + cap, None, :] \
                        .to_broadcast([P, cap, 32, 4])
                    val_b = vrep_t[:, s0 : s0 + cap, None, :] \
                        .to_broadcast([P, cap, 32, 4])
                    nc.vector.tensor_tensor(out=oh[:, :cap, :, :], in0=iota_b,
                                            in1=key_b,
                                            op=mybir.AluOpType.is_equal)
                    nc.vector.tensor_tensor(out=oh[:, :cap, :, :],
                                            in0=oh[:, :cap, :, :], in1=val_b,
                                            op=mybir.AluOpType.mult)
                    oht.append(oh)
                for w in ws:
                    nslots = sum(C[w][q] for q in range(N_QUAD))
                    pw = psw.tile([F, P], dt.float32, tag="pw")
                    j = 0
                    for q in range(N_QUAD):
                        s0 = slot_base[(w, q)]
                        k0 = s0 - slot_base[(ws[0], q)]
                        for k in range(C[w][q]):
                            nc.tensor.matmul(
                                pw[:], lhsT=gt[q][:, k0 + k, :],
                                rhs=oht[q][:, k0 + k, :, :],
                                start=(j == 0), stop=(j == nslots - 1),
                            )
                            j += 1
                    at = sby.tile([F, P], dt.float16, tag="at")
                    nc.scalar.activation(at[:], pw[:],
                                         mybir.ActivationFunctionType.Copy)
                    z = psz.tile([F, P], dt.float32, tag="z")
                    nc.tensor.matmul(z[:], lhsT=w_t[:], rhs=at[:],
                                     start=True, stop=True)
                    ys = sby.tile([F, P], dt.float32, tag="ys")
                    nc.scalar.activation(ys[:], z[:],
                                         mybir.ActivationFunctionType.Sigmoid,
                                         bias=bias_t[:, 0:1])
                    nc.sync.dma_start(yt_d[:, w * P : (w + 1) * P], ys[:])

    nc.compile()
    return nc


def _preprocess(X, edge_row, edge_col, edge_val, weight, bias):
    edge_row = np.asarray(edge_row, dtype=np.int64)
    edge_col = np.asarray(edge_col, dtype=np.int64)
    edge_val = np.asarray(edge_val, dtype=np.float32)

    if not np.all(edge_row[:-1] <= edge_row[1:]):
        o = np.argsort(edge_row, kind="stable")
        edge_row, edge_col, edge_val = edge_row[o], edge_col[o], edge_val[o]

    core = edge_row // ROWS_PER_CORE
    rl = edge_row % ROWS_PER_CORE
    win = rl // P
    dkey = (rl - win * P).astype(np.float16)
    quad = edge_col // QUAD
    cidx = (edge_col % QUAD).astype(np.int16)

    ncells = NCORES * N_WIN * N_QUAD
    cell_id = (core * N_WIN + win) * N_QUAD + quad
    counts = np.bincount(cell_id, minlength=ncells).reshape(NCORES, N_WIN, N_QUAD)

    C = -(-counts.max(axis=0) // P)              # [N_WIN, N_QUAD]
    C = np.maximum(C, 1)

    order_cells = []
    for g in range(N_GRP):
        ws = range(g * GW, min((g + 1) * GW, N_WIN))
        for q in range(N_QUAD):
            for w in ws:
                order_cells.append((w, q))
    slot_base = np.zeros((N_WIN, N_QUAD), dtype=np.int64)
    s = 0
    for (w, q) in order_cells:
        slot_base[w, q] = s
        s += C[w, q]
    S2 = int(s)

    # sort edges by (core, cell, col); col-sort gives the gather descriptors
    # ascending DRAM addresses within each cell
    skey = cell_id * QUAD + (edge_col % QUAD)
    order = np.argsort(skey, kind="stable")
    cell_s = cell_id[order] % (N_WIN * N_QUAD)
    core_s = core[order]
    w_s = cell_s // N_QUAD
    q_s = cell_s % N_QUAD

    cstarts = np.zeros(ncells, dtype=np.int64)
    cstarts[1:] = np.cumsum(counts.reshape(-1))[:-1]
    rank = np.arange(len(order), dtype=np.int64) - cstarts[cell_id[order]]
    pos = slot_base[w_s, q_s] * P + rank         # flat idx position within core

    key_flat = np.zeros((NCORES, S2 * P), dtype=np.float16)
    val_flat = np.zeros((NCORES, S2 * P), dtype=np.float16)
    gidx_flat = np.zeros((NCORES, S2 * P), dtype=np.int16)
    key_flat[core_s, pos] = dkey[order]
    val_flat[core_s, pos] = edge_val[order].astype(np.float16)
    gidx_flat[core_s, pos] = cidx[order]

    # [NC, S2*P] -> [NC, P, S2] -> replicate x4 -> [NC, P, S2, 4]
    key_m = key_flat.reshape(NCORES, S2, P).transpose(0, 2, 1)
    val_m = val_flat.reshape(NCORES, S2, P).transpose(0, 2, 1)
    krep = np.ascontiguousarray(np.repeat(key_m[:, :, :, None], 4, axis=3))
    vrep = np.ascontiguousarray(np.repeat(val_m[:, :, :, None], 4, axis=3))

    gw_ = gidx_flat.reshape(NCORES, S2, 8, 16).transpose(0, 1, 3, 2)
    gw_ = np.tile(gw_, (1, 1, 8, 1))
    gidx_m = np.ascontiguousarray(gw_.transpose(0, 2, 1, 3).reshape(NCORES, P, S2 * 8))

    iota = np.tile(np.arange(P, dtype=np.float16).reshape(1, 32, 4), (P, 1, 1))
    shared = {
        "x": np.ascontiguousarray(np.asarray(X, dtype=np.float16)),
        "w": np.ascontiguousarray(np.asarray(weight, dtype=np.float16)),
        "bias": np.ascontiguousarray(np.asarray(bias, dtype=np.float32)[:, None]),
        "iota": np.ascontiguousarray(iota),
    }
    per_core = [
        {"gidx": gidx_m[c], "krep": krep[c], "vrep": vrep[c]}
        for c in range(NCORES)
    ]
    return C, shared, per_core


def kernel(X, edge_row, edge_col, edge_val, weight, bias):
    C, shared, per_core = _preprocess(X, edge_row, edge_col, edge_val,
                                      weight, bias)
    ckey = C.tobytes()
    if ckey not in _cache:
        _cache[ckey] = _build_program(C)
    nc = _cache[ckey]

    in_maps = [dict(shared, **per_core[c]) for c in range(NCORES)]
    res = bass_utils.run_bass_kernel_spmd(nc, in_maps, core_ids=list(range(NCORES)))

    out = np.empty((N_NODES, F), dtype=np.float32)
    for c in range(NCORES):
        out[c * ROWS_PER_CORE : (c + 1) * ROWS_PER_CORE] = \
            res.results[c]["yt"].T[:ROWS_PER_CORE]
    return out


# revision 13
# speedup vs baseline: 2.7221x; 1.0089x over previous
"""GCNConv Trainium2 kernel: sigmoid(segment_sum(edge_val * (X@W)[edge_col], edge_row) + bias).

Uses the reassociation A@(XW) = (A@X)W:
  - Shard destination rows across 8 cores (12500 rows each); edges partitioned
    by dest row (edge_row is sorted). One NEFF runs SPMD on all 8 cores.
  - Dest rows in 128-row windows (98/core). Gathers batched per (group of 6
    windows, node-quadrant): 68 dma_gather calls/core amortize the ~1us SWDGE
    fixed cost. X cast to fp16 on host: 256B descriptors, 1-cycle/row PE.
  - One-hot build is BATCHED on the DVE: per (group, quadrant) region, ONE
    tensor_tensor is_equal + ONE mult build all slots' val-scaled one-hots.
    Per-slot keys/vals enter via a replicated-x4 fp16 layout [P, S2, 4] read
    through a [cap][32 x stride-0][4 x stride-1] access pattern - last dim
    packed so the DVE 2x_1p fp16 mode can engage (vs 1x for broadcasts).
  - Per 128-edge slot, PE matmul lhsT=gathered-rows, rhs=one-hot accumulates
    the TRANSPOSED window aggregate [feat, 128 dests] in PSUM.
  - Window epilogue: copy PSUM->SBUF fp16, z = matmul(lhsT=W_fp16, rhs), one
    ACT computes sigmoid(z + bias), DMA to transposed [128, 12544] output;
    host transposes once.
"""
import sys

sys.path.insert(0, "/opt/trn_rl_repo")

import numpy as np

import concourse.mybir as mybir
import concourse.tile as tile
from concourse import bacc
import concourse.bass_utils as bass_utils

# Problem constants (contest contract)
N_NODES = 100000
F = 128
NCORES = 8
ROWS_PER_CORE = N_NODES // NCORES          # 12500
P = 128
N_WIN = -(-ROWS_PER_CORE // P)             # 98 windows/core (last covers 84)
GW = 3                                     # windows per gather group
N_GRP = -(-N_WIN // GW)                    # 33 groups
N_QUAD = 4
QUAD = N_NODES // N_QUAD                   # 25000 (< int16 max)
GBUFS = 8                                  # gathered tiles in flight (2 groups)
OHBUFS = 4
NQUEUES = 4

_cache = {}


def _make_groups(C):
    """Greedy window grouping: max per-(group,quadrant) idx count stays under
    the 2048-idx SWDGE ring so descriptor generation never stalls on drain."""
    MAXSLOTS = 15
    groups = []
    cur = []
    acc = [0] * N_QUAD
    for w in range(N_WIN):
        nxt = [acc[q] + C[w][q] for q in range(N_QUAD)]
        if cur and max(nxt) > MAXSLOTS:
            groups.append(cur)
            cur = [w]
            acc = [C[w][q] for q in range(N_QUAD)]
        else:
            cur = cur + [w]
            acc = nxt
    if cur:
        groups.append(cur)
    return groups



def _build_program(C):
    """Build + compile the SPMD program. C: [N_WIN][N_QUAD] slots per cell
    (identical across cores)."""
    dt = mybir.dt
    C = [list(map(int, row)) for row in C]
    S2 = sum(sum(row) for row in C)
    grp_ws = _make_groups(C)
    cap_gq = [[sum(C[w][q] for w in ws) for q in range(N_QUAD)]
              for g, ws in enumerate(grp_ws)]
    CMAX = max(max(row) for row in cap_gq)
    slot_base = {}
    s = 0
    for g, ws in enumerate(grp_ws):
        for q in range(N_QUAD):
            for w in ws:
                slot_base[(w, q)] = s
                s += C[w][q]
    assert s == S2

    nc = bacc.Bacc("TRN2", target_bir_lowering=False, debug=False,
                   enable_asserts=False, num_devices=NCORES,
                   num_swdge_queues=NQUEUES)

    x_d = nc.dram_tensor("x", [N_NODES, F], dt.float16, kind="ExternalInput")
    w_d = nc.dram_tensor("w", [F, F], dt.float16, kind="ExternalInput")
    bias_d = nc.dram_tensor("bias", [F, 1], dt.float32, kind="ExternalInput")
    iota_d = nc.dram_tensor("iota", [P, 32, 4], dt.float16, kind="ExternalInput")
    gidx_d = nc.dram_tensor("gidx", [P, S2 * 8], dt.int16, kind="ExternalInput")
    krep_d = nc.dram_tensor("krep", [P, S2, 4], dt.float16, kind="ExternalInput")
    vrep_d = nc.dram_tensor("vrep", [P, S2, 4], dt.float16, kind="ExternalInput")
    yt_d = nc.dram_tensor("yt", [F, N_WIN * P], dt.float32, kind="ExternalOutput")

    with tile.TileContext(nc) as tc:
        with (
            tc.tile_pool(name="cst", bufs=1) as cst,
            tc.tile_pool(name="sbg", bufs=GBUFS) as sbg,
            tc.tile_pool(name="sbo", bufs=OHBUFS) as sbo,
            tc.tile_pool(name="sby", bufs=4) as sby,
            tc.tile_pool(name="psw", bufs=4, space="PSUM") as psw,
            tc.tile_pool(name="psz", bufs=2, space="PSUM") as psz,
        ):
            iota_t = cst.tile([P, 32, 4], dt.float16)
            nc.sync.dma_start(iota_t[:], iota_d[:])
            w_t = cst.tile([F, F], dt.float16)
            nc.sync.dma_start(w_t[:], w_d[:])
            bias_t = cst.tile([F, 1], dt.float32)
            nc.sync.dma_start(bias_t[:], bias_d[:])
            gidx_t = cst.tile([P, S2 * 8], dt.int16)
            nc.sync.dma_start(gidx_t[:], gidx_d[:])
            krep_t = cst.tile([P, S2, 4], dt.float16)
            nc.sync.dma_start(krep_t[:], krep_d[:])
            vrep_t = cst.tile([P, S2, 4], dt.float16)
            nc.sync.dma_start(vrep_t[:], vrep_d[:])

            qn = 0
            for g, ws in enumerate(grp_ws):
                gt = []
                oht = []
                for q in range(N_QUAD):
                    cap = cap_gq[g][q]
                    s0 = slot_base[(ws[0], q)]
                    gq = sbg.tile([P, CMAX, F], dt.float16, tag="g")
                    nc.gpsimd.dma_gather(
                        out_ap=gq[:, :cap, :],
                        in_ap=x_d[q * QUAD : (q + 1) * QUAD, :],
                        idxs_ap=gidx_t[:, s0 * 8 : (s0 + cap) * 8],
                        num_idxs=cap * P,
                        num_idxs_reg=cap * P,
                        elem_size=F,
                        single_packet=False,
                        queue_num=qn % NQUEUES,
                    )
                    qn += 1
                    gt.append(gq)
                    # batched one-hot build for the whole (group, quadrant)
                    # region: oh[p, k, d] = (iota[d] == key[p, s0+k]) * val
                    oh = sbo.tile([P, CMAX, 32, 4], dt.float16, tag="oh")
                    iota_b = iota_t[:, None, :, :].to_broadcast([P, cap, 32, 4])
                    key_b = krep_t[:, s0 : s0 + cap, None, :] \
                        .to_broadcast([P, cap, 32, 4])
                    val_b = vrep_t[:, s0 : s0 + cap, None, :] \
                        .to_broadcast([P, cap, 32, 4])
                    nc.vector.tensor_tensor(out=oh[:, :cap, :, :], in0=iota_b,
                                            in1=key_b,
                                            op=mybir.AluOpType.is_equal)
                    nc.vector.tensor_tensor(out=oh[:, :cap, :, :],
                                            in0=oh[:, :cap, :, :], in1=val_b,
                                            op=mybir.AluOpType.mult)
                    oht.append(oh)
                for w in ws:
                    nslots = sum(C[w][q] for q in range(N_QUAD))
                    pw = psw.tile([F, P], dt.float32, tag="pw")
                    j = 0
                    for q in range(N_QUAD):
                        s0 = slot_base[(w, q)]
                        k0 = s0 - slot_base[(ws[0], q)]
                        for k in range(C[w][q]):
                            nc.tensor.matmul(
                                pw[:], lhsT=gt[q][:, k0 + k, :],
                                rhs=oht[q][:, k0 + k, :, :],
                                start=(j == 0), stop=(j == nslots - 1),
                            )
                            j += 1
                    at = sby.tile([F, P], dt.float16, tag="at")
                    nc.scalar.activation(at[:], pw[:],
                                         mybir.ActivationFunctionType.Copy)
                    z = psz.tile([F, P], dt.float32, tag="z")
                    nc.tensor.matmul(z[:], lhsT=w_t[:], rhs=at[:],
                                     start=True, stop=True)
                    ys = sby.tile([F, P], dt.float32, tag="ys")
                    nc.scalar.activation(ys[:], z[:],
                                         mybir.ActivationFunctionType.Sigmoid,
                                         bias=bias_t[:, 0:1])
                    nc.sync.dma_start(yt_d[:, w * P : (w + 1) * P], ys[:])

    nc.compile()
    return nc


def _preprocess(X, edge_row, edge_col, edge_val, weight, bias):
    edge_row = np.asarray(edge_row, dtype=np.int64)
    edge_col = np.asarray(edge_col, dtype=np.int64)
    edge_val = np.asarray(edge_val, dtype=np.float32)

    if not np.all(edge_row[:-1] <= edge_row[1:]):
        o = np.argsort(edge_row, kind="stable")
        edge_row, edge_col, edge_val = edge_row[o], edge_col[o], edge_val[o]

    core = edge_row // ROWS_PER_CORE
    rl = edge_row % ROWS_PER_CORE
    win = rl // P
    dkey = (rl - win * P).astype(np.float16)
    quad = edge_col // QUAD
    cidx = (edge_col % QUAD).astype(np.int16)

    ncells = NCORES * N_WIN * N_QUAD
    cell_id = (core * N_WIN + win) * N_QUAD + quad
    counts = np.bincount(cell_id, minlength=ncells).reshape(NCORES, N_WIN, N_QUAD)

    C = -(-counts.max(axis=0) // P)              # [N_WIN, N_QUAD]
    C = np.maximum(C, 1)

    order_cells = []
    for ws in _make_groups(C):
        for q in range(N_QUAD):
            for w in ws:
                order_cells.append((w, q))
    slot_base = np.zeros((N_WIN, N_QUAD), dtype=np.int64)
    s = 0
    for (w, q) in order_cells:
        slot_base[w, q] = s
        s += C[w, q]
    S2 = int(s)

    # sort edges by (core, cell, col); col-sort gives the gather descriptors
    # ascending DRAM addresses within each cell
    skey = cell_id * QUAD + (edge_col % QUAD)
    order = np.argsort(skey, kind="stable")
    cell_s = cell_id[order] % (N_WIN * N_QUAD)
    core_s = core[order]
    w_s = cell_s // N_QUAD
    q_s = cell_s % N_QUAD

    cstarts = np.zeros(ncells, dtype=np.int64)
    cstarts[1:] = np.cumsum(counts.reshape(-1))[:-1]
    rank = np.arange(len(order), dtype=np.int64) - cstarts[cell_id[order]]
    pos = slot_base[w_s, q_s] * P + rank         # flat idx position within core

    key_flat = np.zeros((NCORES, S2 * P), dtype=np.float16)
    val_flat = np.zeros((NCORES, S2 * P), dtype=np.float16)
    gidx_flat = np.zeros((NCORES, S2 * P), dtype=np.int16)
    key_flat[core_s, pos] = dkey[order]
    val_flat[core_s, pos] = edge_val[order].astype(np.float16)
    gidx_flat[core_s, pos] = cidx[order]

    # [NC, S2*P] -> [NC, P, S2] -> replicate x4 -> [NC, P, S2, 4]
    key_m = key_flat.reshape(NCORES, S2, P).transpose(0, 2, 1)
    val_m = val_flat.reshape(NCORES, S2, P).transpose(0, 2, 1)
    krep = np.ascontiguousarray(np.repeat(key_m[:, :, :, None], 4, axis=3))
    vrep = np.ascontiguousarray(np.repeat(val_m[:, :, :, None], 4, axis=3))

    gw_ = gidx_flat.reshape(NCORES, S2, 8, 16).transpose(0, 1, 3, 2)
    gw_ = np.tile(gw_, (1, 1, 8, 1))
    gidx_m = np.ascontiguousarray(gw_.transpose(0, 2, 1, 3).reshape(NCORES, P, S2 * 8))

    iota = np.tile(np.arange(P, dtype=np.float16).reshape(1, 32, 4), (P, 1, 1))
    shared = {
        "x": np.ascontiguousarray(np.asarray(X, dtype=np.float16)),
        "w": np.ascontiguousarray(np.asarray(weight, dtype=np.float16)),
        "bias": np.ascontiguousarray(np.asarray(bias, dtype=np.float32)[:, None]),
        "iota": np.ascontiguousarray(iota),
    }
    per_core = [
        {"gidx": gidx_m[c], "krep": krep[c], "vrep": vrep[c]}
        for c in range(NCORES)
    ]
    return C, shared, per_core


def kernel(X, edge_row, edge_col, edge_val, weight, bias):
    C, shared, per_core = _preprocess(X, edge_row, edge_col, edge_val,
                                      weight, bias)
    ckey = C.tobytes()
    if ckey not in _cache:
        _cache[ckey] = _build_program(C)
    nc = _cache[ckey]

    in_maps = [dict(shared, **per_core[c]) for c in range(NCORES)]
    res = bass_utils.run_bass_kernel_spmd(nc, in_maps, core_ids=list(range(NCORES)))

    out = np.empty((N_NODES, F), dtype=np.float32)
    for c in range(NCORES):
        out[c * ROWS_PER_CORE : (c + 1) * ROWS_PER_CORE] = \
            res.results[c]["yt"].T[:ROWS_PER_CORE]
    return out
